# revision 17
# baseline (speedup 1.0000x reference)
"""Trainium2 Bass kernel for nn_Attention1D (B=4, L=4096, C=64).

reference:
    Q = x@Wq + bq ; K = x@Wk + bk ; V = x@Wv + bv          (per batch b)
    s = Q @ K.T / sqrt(C)                                   [L_q, L_k]
    attn = softmax(s, axis=q)      # normalize over QUERY axis
    out = attn @ V + x

Sharding: 8 cores = 4 batches x 2 key-shards (k in [0,2048) / [2048,4096)).
softmax normalizes over q (not sharded) -> per-core softmax fully local:
    Z[k]   = sum_q exp(s[q,k]);  out_qf = sum_k exp(s[q,k]) * (V[k,f]/Z[k])
k-shards' partial outputs ADD on the host (+ residual x).

Design (exp-wall split across ScalarE+VectorE, PE restructured vs v1):
  - scores transposed sT[k,q], channel-major. Per k-tile (128 keys): 4 gens
    of [128,1024] PSUM slots (2-buf ring, 4 banks), each filled by a
    row-packed MM pair (two 512-q chunks concurrently in PE rows 0-63/64-127
    via doubled Q/K channel copies, QT/KT in bf16: per-k score offsets cancel
    in the softmax-over-q ratio, measured 5e-4 total).
  - exp split per k-tile: ScalarE ACT-Exp on q[0, 2048+SS) (free Z via
    accum_out); VectorE does q[2048+SS, 4096) with the Schraudolph int16
    bit-trick: i16 = round(s*A + B) bitcast as bf16 == exp(s)*(1+-4%); its Z
    via one tensor_reduce. A = 128/ln2 is folded into Wq host-side, the ACT
    uses scale=ln2/128 to undo it. Trick error washes out in the softmax
    ratio + 4096-key sum + residual.
  - AV: outT[f,q] PSUM-accumulated with gv = V/Z STATIONARY (64-col
    LDWEIGHTS): col-packed pairs (even k-tile -> PE cols 0-63 -> acc rows
    0:64, odd -> 64:128; per-region start=True). acc0 [128,2048] (4 banks)
    covers q[0,2048) in-loop (AV deferred behind gv); q[2048,4096) after the
    loop in the freed score banks.
  - HEAT dummy matmuls per gen keep the PE HAM at K=8/8 (cleared by the real
    MM's start=True); gv on GPSIMD; output staged bf16, host does
    out.T = o[h][0:64]+o[h][64:128], + partner core + residual.
"""

import numpy as np
import ml_dtypes  # noqa: F401

B, L, C = 4, 4096, 64
NCORES = 8
KSH = L // 2          # keys per core: 2048
NKT = KSH // 128      # 16 k-tiles per core
SS = 640              # ScalarE's share of gen2's 1024 cols (tunable)
HEAT = 0              # heater MMs per score gen (HAM K=8/8 keepalive)
GV_GPSIMD = False     # compute gv = V*rz on GPSIMD (else VectorE)
AEXP = 128.0 / np.log(2.0)          # folded into Wq
BOFF = 16256.0 - 7.42               # int16 exp bias (round-to-nearest HW)

_cache = {}


def _build():
    import concourse.bacc as bacc
    import concourse.mybir as mybir
    import concourse.tile as tile
    from concourse.bass import _add_dep_helper

    bf16 = mybir.dt.bfloat16
    f32 = mybir.dt.float32
    f32r = mybir.dt.float32r
    i16 = mybir.dt.int16
    AF = mybir.ActivationFunctionType
    ALU = mybir.AluOpType
    AX = mybir.AxisListType

    nc = bacc.Bacc("TRN2", target_bir_lowering=False, debug=False)

    xt_d = nc.dram_tensor("xt", [C + 1, L], f32r, kind="ExternalInput")
    xk_d = nc.dram_tensor("xk", [C + 1, KSH], f32r, kind="ExternalInput")
    wq_d = nc.dram_tensor("wq", [C + 1, 2 * C], f32r, kind="ExternalInput")
    wk_d = nc.dram_tensor("wk", [C + 1, 2 * C], f32r, kind="ExternalInput")
    wv_d = nc.dram_tensor("wv", [C + 1, C], f32r, kind="ExternalInput")
    o_d = nc.dram_tensor("o", [2, 128, KSH], bf16, kind="ExternalOutput")

    with tile.TileContext(nc) as tc:
        with (
            tc.tile_pool(name="consts", bufs=1) as consts,
            tc.tile_pool(name="sb", bufs=1) as sb,
            tc.tile_pool(name="scp", bufs=2, space="PSUM") as scp,
            tc.tile_pool(name="accp", bufs=1, space="PSUM") as accp,
        ):
            acc0 = accp.tile([128, 2048], f32, tag="acc")   # 4 banks

            # --- HAM warmup: dummy matmuls through the slot ring ---
            wu = consts.tile([128, 512], bf16)
            nc.vector.memset(wu, 0.0)
            for _ in range(12):
                wps = scp.tile([128, 1024], f32, tag="slot")
                nc.tensor.matmul(wps[:, 0:512], lhsT=wu[:, 0:128], rhs=wu,
                                 start=True, stop=True)

            wq_s = consts.tile([C + 1, 2 * C], f32r)
            wk_s = consts.tile([C + 1, 2 * C], f32r)
            wv_s = consts.tile([C + 1, C], f32r)
            nc.sync.dma_start(out=wq_s, in_=wq_d.ap())
            nc.sync.dma_start(out=wk_s, in_=wk_d.ap())
            nc.sync.dma_start(out=wv_s, in_=wv_d.ap())

            xt_c = []
            for c in range(8):
                t = sb.tile([C + 1, 512], f32r, tag=f"xt{c}")
                nc.sync.dma_start(out=t, in_=xt_d.ap()[:, c * 512:(c + 1) * 512])
                xt_c.append(t)
            xk_c = []
            for c in range(4):
                t = sb.tile([C + 1, 512], f32r, tag=f"xk{c}")
                nc.sync.dma_start(out=t, in_=xk_d.ap()[:, c * 512:(c + 1) * 512])
                xk_c.append(t)

            # persistent SBUF state
            qt_s = sb.tile([128, 8, 512], bf16, tag="qt")
            kt2 = sb.tile([128, 4, 512], bf16, tag="kt")   # [2C, k-half]
            v_sb = sb.tile([128, NKT, C], bf16, tag="v")
            gv_all = sb.tile([128, NKT, C], bf16, tag="gv")
            e_all = sb.tile([128, NKT, L], bf16, tag="e")
            zps = sb.tile([128, NKT, 4], f32, tag="zps")
            zz = sb.tile([128, NKT], f32, tag="zz")
            rz = sb.tile([128, NKT], f32, tag="rz")
            dume = sb.tile([128, 1], bf16, tag="dume")

            # force the exp table set before any Copy-ACT evacuations
            nc.scalar.activation(out=dume, in_=wu[:, 0:1], func=AF.Exp)

            # --- projections through the slot ring, paired 512-chunks ---
            for g in range(4):
                slot = scp.tile([128, 1024], f32, tag="slot")
                for h in range(2):
                    nc.tensor.matmul(
                        slot[:, h * 512:(h + 1) * 512], lhsT=wq_s,
                        rhs=xt_c[2 * g + h], start=True, stop=True,
                    )
                if g % 2 == 0:
                    nc.scalar.activation(out=qt_s[:, 2 * g:2 * g + 2, :],
                                         in_=slot, func=AF.Copy)
                else:
                    nc.vector.tensor_copy(out=qt_s[:, 2 * g:2 * g + 2, :],
                                          in_=slot)
            for g in range(2):
                slot = scp.tile([128, 1024], f32, tag="slot")
                for h in range(2):
                    nc.tensor.matmul(
                        slot[:, h * 512:(h + 1) * 512], lhsT=wk_s,
                        rhs=xk_c[2 * g + h], start=True, stop=True,
                    )
                if g == 0:
                    nc.scalar.activation(out=kt2[:, 0:2, :], in_=slot,
                                         func=AF.Copy)
                else:
                    nc.vector.tensor_copy(out=kt2[:, 2:4, :], in_=slot)
            # V: one gen holds all 16 k-tiles' [128,64]
            vslot = scp.tile([128, 1024], f32, tag="slot")
            for kt in range(NKT):
                nc.tensor.matmul(
                    vslot[:, kt * C:(kt + 1) * C],
                    lhsT=xk_c[kt // 4][:, (kt % 4) * 128:(kt % 4 + 1) * 128],
                    rhs=wv_s, start=True, stop=True,
                )
            nc.vector.tensor_copy(out=v_sb, in_=vslot)

            def kslice(kt, r0, r1):
                return kt2[r0:r1, kt // 4, (kt % 4) * 128:(kt % 4 + 1) * 128]

            # --- main loop over k-pairs ---
            av_queue = []   # deferred phase-0 AV emitters

            def emit_av_pair(p):
                # phase-0 AV for pair p: q[0,2048) in 4 chunks, col-packed
                ke, ko = 2 * p, 2 * p + 1
                ops = []
                for cq in range(4):
                    def mk(cq=cq, ke=ke, ko=ko, p=p):
                        me = nc.tensor.matmul(
                            acc0[0:64, cq * 512:(cq + 1) * 512],
                            lhsT=gv_all[:, ke, :],
                            rhs=e_all[:, ke, cq * 512:(cq + 1) * 512],
                            tile_position=(0, 0),
                            start=(p == 0), stop=(p == 7),
                            skip_group_check=True,
                        )
                        mo = nc.tensor.matmul(
                            acc0[64:128, cq * 512:(cq + 1) * 512],
                            lhsT=gv_all[:, ko, :],
                            rhs=e_all[:, ko, cq * 512:(cq + 1) * 512],
                            tile_position=(0, 64),
                            start=(p == 0), stop=(p == 7),
                            skip_group_check=True,
                        )
                        _add_dep_helper(mo.ins, me.ins, sync=False,
                                        reason="av pair order")
                    ops.append(mk)
                return ops

            def drain_av(n):
                for _ in range(n):
                    if av_queue:
                        av_queue.pop(0)()

            for p in range(8):
                for kt in (2 * p, 2 * p + 1):
                    lA = kslice(kt, 0, C)
                    lB = kslice(kt, C, 128)
                    for g in range(4):
                        slot = scp.tile([128, 1024], f32, tag="slot")
                        # heaters keep HAM busy; cleared by real start=True
                        for _ in range(HEAT):
                            nc.tensor.matmul(slot[:, 0:512],
                                             lhsT=wu[:, 0:128], rhs=wu,
                                             start=True, stop=True)
                        qc = 2 * g
                        ma = nc.tensor.matmul(
                            slot[:, 0:512], lhsT=lA,
                            rhs=qt_s[0:C, qc, :], tile_position=(0, 0),
                            start=True, stop=True,
                        )
                        mb = nc.tensor.matmul(
                            slot[:, 512:1024], lhsT=lB,
                            rhs=qt_s[C:128, qc + 1, :], tile_position=(C, 0),
                            start=True, stop=True,
                        )
                        _add_dep_helper(mb.ins, ma.ins, sync=False,
                                        reason="score pair order")
                        q0 = g * 1024
                        if g < 2:
                            nc.scalar.activation(
                                out=e_all[:, kt, q0:q0 + 1024], in_=slot,
                                func=AF.Exp, scale=float(np.log(2.0) / 128.0),
                                accum_out=zps[:, kt, g:g + 1],
                            )
                        elif g == 2:
                            nc.scalar.activation(
                                out=e_all[:, kt, q0:q0 + SS], in_=slot[:, 0:SS],
                                func=AF.Exp, scale=float(np.log(2.0) / 128.0),
                                accum_out=zps[:, kt, 2:3],
                            )
                            nc.vector.tensor_scalar(
                                out=e_all[:, kt, q0 + SS:q0 + 1024].bitcast(i16),
                                in0=slot[:, SS:1024], scalar1=BOFF,
                                scalar2=None, op0=ALU.add,
                            )
                        else:
                            nc.vector.tensor_scalar(
                                out=e_all[:, kt, q0:q0 + 1024].bitcast(i16),
                                in0=slot, scalar1=BOFF,
                                scalar2=None, op0=ALU.add,
                            )
                        drain_av(1)
                    # Z for the DVE range
                    nc.vector.tensor_reduce(
                        out=zps[:, kt, 3:4], in_=e_all[:, kt, 2048 + SS:L],
                        axis=AX.X, op=ALU.add,
                    )
                if p % 2 == 1:
                    # z-combine + reciprocal + gv for k-tiles 4*(p//2)..+4
                    j = 4 * (p // 2)
                    nc.vector.tensor_reduce(
                        out=zz[:, j:j + 4], in_=zps[:, j:j + 4, :],
                        axis=AX.X, op=ALU.add,
                    )
                    nc.vector.reciprocal(out=rz[:, j:j + 4], in_=zz[:, j:j + 4])
                    for kt in range(j, j + 4):
                        if GV_GPSIMD:
                            nc.gpsimd.tensor_scalar(
                                out=gv_all[:, kt, :], in0=v_sb[:, kt, :],
                                scalar1=rz[:, kt:kt + 1], scalar2=None,
                                op0=ALU.mult,
                            )
                        else:
                            nc.vector.tensor_scalar_mul(
                                gv_all[:, kt, :], v_sb[:, kt, :],
                                rz[:, kt:kt + 1]
                            )
                    # their gv is ready: enqueue AV for pairs p-1, p
                    av_queue.extend(emit_av_pair(p - 1))
                    av_queue.extend(emit_av_pair(p))

            # drain remaining phase-0 AV (pairs 6,7)
            drain_av(len(av_queue))

            ob0 = sb.tile([128, 2048], bf16, tag="ob0")
            nc.scalar.activation(out=ob0, in_=acc0, func=AF.Copy)
            nc.sync.dma_start(out=o_d.ap()[0], in_=ob0)

            # --- phase 1: q[2048,4096) in freed score banks ---
            acc1a = scp.tile([128, 1024], f32, tag="slot")
            acc1b = scp.tile([128, 1024], f32, tag="slot")
            for half, acc1 in ((0, acc1a), (1, acc1b)):
                for cq in range(2):
                    qg = 2048 + half * 1024 + cq * 512
                    for p in range(8):
                        ke, ko = 2 * p, 2 * p + 1
                        me = nc.tensor.matmul(
                            acc1[0:64, cq * 512:(cq + 1) * 512],
                            lhsT=gv_all[:, ke, :],
                            rhs=e_all[:, ke, qg:qg + 512],
                            tile_position=(0, 0),
                            start=(p == 0), stop=(p == 7),
                            skip_group_check=True,
                        )
                        mo = nc.tensor.matmul(
                            acc1[64:128, cq * 512:(cq + 1) * 512],
                            lhsT=gv_all[:, ko, :],
                            rhs=e_all[:, ko, qg:qg + 512],
                            tile_position=(0, 64),
                            start=(p == 0), stop=(p == 7),
                            skip_group_check=True,
                        )
                        _add_dep_helper(mo.ins, me.ins, sync=False,
                                        reason="av1 pair order")
                ob1 = sb.tile([128, 1024], bf16, tag=f"ob1{half}")
                nc.scalar.activation(out=ob1, in_=acc1, func=AF.Copy)
                nc.sync.dma_start(
                    out=o_d.ap()[1][:, half * 1024:(half + 1) * 1024],
                    in_=ob1,
                )

    nc.compile()
    return nc


def _get_nc():
    if "nc" not in _cache:
        _cache["nc"] = _build()
    return _cache["nc"]


def _in_maps(x, Wq, bq, Wk, bk, Wv, bv):
    s = np.float32(AEXP / np.sqrt(np.float32(C)))
    wq1 = (np.concatenate([Wq, bq[None, :]], 0) * s).astype(np.float32)
    wq1 = np.concatenate([wq1, wq1], 1)          # doubled -> replicated QT
    wk1 = np.concatenate([Wk, bk[None, :]], 0).astype(np.float32)
    wk1 = np.concatenate([wk1, wk1], 1)
    wv1 = np.concatenate([Wv, bv[None, :]], 0).astype(np.float32)
    maps = []
    for core in range(NCORES):
        b, half = core // 2, core % 2
        x1t = np.ascontiguousarray(np.concatenate(
            [x[b], np.ones((L, 1), np.float32)], 1
        ).T.astype(np.float32))              # [65, L]
        xk = np.ascontiguousarray(x1t[:, half * KSH:(half + 1) * KSH])
        maps.append({
            "xt": x1t,
            "xk": xk,
            "wq": wq1, "wk": wk1, "wv": wv1,
        })
    return maps


def _assemble(outs, x):
    full = np.empty((B, L, C), np.float32)
    for b in range(B):
        o0, o1 = outs[2 * b], outs[2 * b + 1]
        att = (o0[0, 0:64] + o0[0, 64:128] + o1[0, 0:64] + o1[0, 64:128],
               o0[1, 0:64] + o0[1, 64:128] + o1[1, 0:64] + o1[1, 64:128])
        full[b] = np.concatenate(att, axis=1).T + x[b]
    return full


def _run(x, Wq, bq, Wk, bk, Wv, bv, trace=False):
    from concourse.bass_utils import run_bass_kernel_spmd

    nc = _get_nc()
    maps = _in_maps(x, Wq, bq, Wk, bk, Wv, bv)
    res = run_bass_kernel_spmd(
        nc, maps, core_ids=list(range(NCORES)), trace=trace
    )
    outs = [r["o"].astype(np.float32) for r in res.results]
    return _assemble(outs, x), res


def kernel(x, Wq, bq, Wk, bk, Wv, bv):
    x = np.asarray(x, np.float32)
    full, _ = _run(
        x,
        np.asarray(Wq, np.float32), np.asarray(bq, np.float32),
        np.asarray(Wk, np.float32), np.asarray(bk, np.float32),
        np.asarray(Wv, np.float32), np.asarray(bv, np.float32),
    )
    return full


# revision 18
# speedup vs baseline: 1.1204x; 1.1204x over previous
"""Trainium2 Bass kernel for nn_Attention1D (B=4, L=4096, C=64).

reference:
    Q = x@Wq + bq ; K = x@Wk + bk ; V = x@Wv + bv          (per batch b)
    s = Q @ K.T / sqrt(C)                                   [L_q, L_k]
    attn = softmax(s, axis=q)      # normalize over QUERY axis
    out = attn @ V + x

Sharding: 8 cores = 4 batches x 2 key-shards (k in [0,2048) / [2048,4096)).
softmax normalizes over q (not sharded) -> per-core softmax fully local:
    Z[k]   = sum_q exp(s[q,k]);  out_qf = sum_k exp(s[q,k]) * (V[k,f]/Z[k])
k-shards' partial outputs ADD on the host (+ residual x).

Design (exp-wall split across ScalarE+VectorE, PE restructured vs v1):
  - scores transposed sT[k,q], channel-major. Per k-tile (128 keys): 4 gens
    of [128,1024] PSUM slots (2-buf ring, 4 banks), each filled by a
    row-packed MM pair (two 512-q chunks concurrently in PE rows 0-63/64-127
    via doubled Q/K channel copies, QT/KT in bf16: per-k score offsets cancel
    in the softmax-over-q ratio, measured 5e-4 total).
  - exp split per k-tile: ScalarE ACT-Exp on q[0, 2048+SS) (free Z via
    accum_out); VectorE does q[2048+SS, 4096) with the Schraudolph int16
    bit-trick: i16 = round(s*A + B) bitcast as bf16 == exp(s)*(1+-4%); its Z
    via one tensor_reduce. A = 128/ln2 is folded into Wq host-side, the ACT
    uses scale=ln2/128 to undo it. Trick error washes out in the softmax
    ratio + 4096-key sum + residual.
  - AV: outT[f,q] PSUM-accumulated with gv = V/Z STATIONARY (64-col
    LDWEIGHTS): col-packed pairs (even k-tile -> PE cols 0-63 -> acc rows
    0:64, odd -> 64:128; per-region start=True). acc0 [128,2048] (4 banks)
    covers q[0,2048) in-loop (AV deferred behind gv); q[2048,4096) after the
    loop in the freed score banks.
  - HEAT dummy matmuls per gen keep the PE HAM at K=8/8 (cleared by the real
    MM's start=True); gv on GPSIMD; output staged bf16, host does
    out.T = o[h][0:64]+o[h][64:128], + partner core + residual.
"""

import numpy as np
import ml_dtypes  # noqa: F401

B, L, C = 4, 4096, 64
NCORES = 8
KSH = L // 2          # keys per core: 2048
NKT = KSH // 128      # 16 k-tiles per core
SS = 640              # ScalarE's share of gen2's 1024 cols (tunable)
HEAT = 0              # heater MMs per score gen (HAM K=8/8 keepalive)
GV_GPSIMD = True      # compute gv = V*rz on GPSIMD (else VectorE)
AEXP = 128.0 / np.log(2.0)          # folded into Wq
BOFF = 16256.0 - 7.42               # int16 exp bias (round-to-nearest HW)

_cache = {}


def _build():
    import concourse.bacc as bacc
    import concourse.mybir as mybir
    import concourse.tile as tile
    from concourse.bass import _add_dep_helper

    bf16 = mybir.dt.bfloat16
    f32 = mybir.dt.float32
    f32r = mybir.dt.float32r
    i16 = mybir.dt.int16
    AF = mybir.ActivationFunctionType
    ALU = mybir.AluOpType
    AX = mybir.AxisListType

    nc = bacc.Bacc("TRN2", target_bir_lowering=False, debug=False)

    xt_d = nc.dram_tensor("xt", [C + 1, L], f32r, kind="ExternalInput")
    xk_d = nc.dram_tensor("xk", [C + 1, KSH], f32r, kind="ExternalInput")
    wq_d = nc.dram_tensor("wq", [C + 1, 2 * C], f32r, kind="ExternalInput")
    wk_d = nc.dram_tensor("wk", [C + 1, 2 * C], f32r, kind="ExternalInput")
    wv_d = nc.dram_tensor("wv", [C + 1, C], f32r, kind="ExternalInput")
    o_d = nc.dram_tensor("o", [2, 128, KSH], bf16, kind="ExternalOutput")

    with tile.TileContext(nc) as tc:
        with (
            tc.tile_pool(name="consts", bufs=1) as consts,
            tc.tile_pool(name="sb", bufs=1) as sb,
            tc.tile_pool(name="scp", bufs=2, space="PSUM") as scp,
            tc.tile_pool(name="accp", bufs=1, space="PSUM") as accp,
        ):
            acc0 = accp.tile([128, 2048], f32, tag="acc")   # 4 banks

            # --- input DMAs first (sync + gpsimd queues in parallel) ---
            wq_s = consts.tile([C + 1, 2 * C], f32r)
            wk_s = consts.tile([C + 1, 2 * C], f32r)
            wv_s = consts.tile([C + 1, C], f32r)
            nc.sync.dma_start(out=wq_s, in_=wq_d.ap())
            nc.gpsimd.dma_start(out=wk_s, in_=wk_d.ap())
            nc.gpsimd.dma_start(out=wv_s, in_=wv_d.ap())

            xt_c = []
            for c in range(8):
                t = sb.tile([C + 1, 512], f32r, tag=f"xt{c}")
                eng = nc.sync if c % 2 == 0 else nc.gpsimd
                eng.dma_start(out=t, in_=xt_d.ap()[:, c * 512:(c + 1) * 512])
                xt_c.append(t)
            xk_c = []
            for c in range(4):
                t = sb.tile([C + 1, 512], f32r, tag=f"xk{c}")
                eng = nc.sync if c % 2 == 0 else nc.gpsimd
                eng.dma_start(out=t, in_=xk_d.ap()[:, c * 512:(c + 1) * 512])
                xk_c.append(t)

            # --- HAM warmup: dummy matmuls through the slot ring ---
            wu = consts.tile([128, 512], bf16)
            nc.vector.memset(wu, 0.0)
            for _ in range(12):
                wps = scp.tile([128, 1024], f32, tag="slot")
                nc.tensor.matmul(wps[:, 0:512], lhsT=wu[:, 0:128], rhs=wu,
                                 start=True, stop=True)

            # persistent SBUF state
            qt_s = sb.tile([128, 8, 512], bf16, tag="qt")
            kt2 = sb.tile([128, 4, 512], bf16, tag="kt")   # [2C, k-half]
            v_sb = sb.tile([128, NKT, C], bf16, tag="v")
            gv_all = sb.tile([128, NKT, C], bf16, tag="gv")
            e_all = sb.tile([128, NKT * L], bf16, tag="e")
            zps = sb.tile([128, NKT, 4], f32, tag="zps")
            zz = sb.tile([128, NKT], f32, tag="zz")
            rz = sb.tile([128, NKT], f32, tag="rz")
            dume = sb.tile([128, 1], bf16, tag="dume")

            # force the exp table set before any Copy-ACT evacuations
            nc.scalar.activation(out=dume, in_=wu[:, 0:1], func=AF.Exp)

            # --- projections through the slot ring, paired 512-chunks ---
            for g in range(4):
                slot = scp.tile([128, 1024], f32, tag="slot")
                for h in range(2):
                    nc.tensor.matmul(
                        slot[:, h * 512:(h + 1) * 512], lhsT=wq_s,
                        rhs=xt_c[2 * g + h], start=True, stop=True,
                    )
                if g % 2 == 0:
                    nc.scalar.activation(out=qt_s[:, 2 * g:2 * g + 2, :],
                                         in_=slot, func=AF.Copy)
                else:
                    nc.vector.tensor_copy(out=qt_s[:, 2 * g:2 * g + 2, :],
                                          in_=slot)
            for g in range(2):
                slot = scp.tile([128, 1024], f32, tag="slot")
                for h in range(2):
                    nc.tensor.matmul(
                        slot[:, h * 512:(h + 1) * 512], lhsT=wk_s,
                        rhs=xk_c[2 * g + h], start=True, stop=True,
                    )
                if g == 0:
                    nc.scalar.activation(out=kt2[:, 0:2, :], in_=slot,
                                         func=AF.Copy)
                else:
                    nc.vector.tensor_copy(out=kt2[:, 2:4, :], in_=slot)
            # V: one gen holds all 16 k-tiles' [128,64]
            vslot = scp.tile([128, 1024], f32, tag="slot")
            for kt in range(NKT):
                nc.tensor.matmul(
                    vslot[:, kt * C:(kt + 1) * C],
                    lhsT=xk_c[kt // 4][:, (kt % 4) * 128:(kt % 4 + 1) * 128],
                    rhs=wv_s, start=True, stop=True,
                )
            nc.vector.tensor_copy(out=v_sb, in_=vslot)

            def kslice(kt, r0, r1):
                return kt2[r0:r1, kt // 4, (kt % 4) * 128:(kt % 4 + 1) * 128]

            # --- main loop over k-pairs ---
            av_queue = []   # deferred phase-0 AV emitters

            def emit_av_pair(p):
                # phase-0 AV for pair p: q[0,2048) in 4 chunks, col-packed
                ke, ko = 2 * p, 2 * p + 1
                ops = []
                for cq in range(4):
                    def mk(cq=cq, ke=ke, ko=ko, p=p):
                        me = nc.tensor.matmul(
                            acc0[0:64, cq * 512:(cq + 1) * 512],
                            lhsT=gv_all[:, ke, :],
                            rhs=e_all[:, ke * L + cq * 512:ke * L + (cq + 1) * 512],
                            tile_position=(0, 0),
                            start=(p == 0), stop=(p == 7),
                            skip_group_check=True,
                        )
                        mo = nc.tensor.matmul(
                            acc0[64:128, cq * 512:(cq + 1) * 512],
                            lhsT=gv_all[:, ko, :],
                            rhs=e_all[:, ko * L + cq * 512:ko * L + (cq + 1) * 512],
                            tile_position=(0, 64),
                            start=(p == 0), stop=(p == 7),
                            skip_group_check=True,
                        )
                        _add_dep_helper(mo.ins, me.ins, sync=False,
                                        reason="av pair order")
                    ops.append(mk)
                return ops

            def drain_av(n):
                for _ in range(n):
                    if av_queue:
                        av_queue.pop(0)()

            for p in range(8):
                for kt in (2 * p, 2 * p + 1):
                    lA = kslice(kt, 0, C)
                    lB = kslice(kt, C, 128)
                    for g in range(4):
                        slot = scp.tile([128, 1024], f32, tag="slot")
                        # heaters keep HAM busy; cleared by real start=True
                        for _ in range(HEAT):
                            nc.tensor.matmul(slot[:, 0:512],
                                             lhsT=wu[:, 0:128], rhs=wu,
                                             start=True, stop=True)
                        qc = 2 * g
                        ma = nc.tensor.matmul(
                            slot[:, 0:512], lhsT=lA,
                            rhs=qt_s[0:C, qc, :], tile_position=(0, 0),
                            start=True, stop=True,
                        )
                        mb = nc.tensor.matmul(
                            slot[:, 512:1024], lhsT=lB,
                            rhs=qt_s[C:128, qc + 1, :], tile_position=(C, 0),
                            start=True, stop=True,
                        )
                        _add_dep_helper(mb.ins, ma.ins, sync=False,
                                        reason="score pair order")
                        q0 = g * 1024
                        if g < 2:
                            nc.scalar.activation(
                                out=e_all[:, kt * L + q0:kt * L + q0 + 1024], in_=slot,
                                func=AF.Exp, scale=float(np.log(2.0) / 128.0),
                                accum_out=zps[:, kt, g:g + 1],
                            )
                        elif g == 2:
                            nc.scalar.activation(
                                out=e_all[:, kt * L + q0:kt * L + q0 + SS], in_=slot[:, 0:SS],
                                func=AF.Exp, scale=float(np.log(2.0) / 128.0),
                                accum_out=zps[:, kt, 2:3],
                            )
                            nc.vector.tensor_scalar(
                                out=e_all[:, kt * L + q0 + SS:kt * L + q0 + 1024].bitcast(i16),
                                in0=slot[:, SS:1024], scalar1=BOFF,
                                scalar2=None, op0=ALU.add,
                            )
                        else:
                            nc.vector.tensor_scalar(
                                out=e_all[:, kt * L + q0:kt * L + q0 + 1024].bitcast(i16),
                                in0=slot, scalar1=BOFF,
                                scalar2=None, op0=ALU.add,
                            )
                        drain_av(1)
                    # Z for the DVE range
                    nc.vector.tensor_reduce(
                        out=zps[:, kt, 3:4], in_=e_all[:, kt * L + 2048 + SS:(kt + 1) * L],
                        axis=AX.X, op=ALU.add,
                    )
                if p % 2 == 1:
                    # z-combine + reciprocal + gv for k-tiles 4*(p//2)..+4
                    j = 4 * (p // 2)
                    nc.vector.tensor_reduce(
                        out=zz[:, j:j + 4], in_=zps[:, j:j + 4, :],
                        axis=AX.X, op=ALU.add,
                    )
                    nc.vector.reciprocal(out=rz[:, j:j + 4], in_=zz[:, j:j + 4])
                    for kt in range(j, j + 4):
                        if GV_GPSIMD:
                            nc.gpsimd.tensor_scalar(
                                out=gv_all[:, kt, :], in0=v_sb[:, kt, :],
                                scalar1=rz[:, kt:kt + 1], scalar2=None,
                                op0=ALU.mult,
                            )
                        else:
                            nc.vector.tensor_scalar_mul(
                                gv_all[:, kt, :], v_sb[:, kt, :],
                                rz[:, kt:kt + 1]
                            )
                    # their gv is ready: enqueue AV for pairs p-1, p
                    av_queue.extend(emit_av_pair(p - 1))
                    av_queue.extend(emit_av_pair(p))

            # drain remaining phase-0 AV (pairs 6,7)
            drain_av(len(av_queue))

            ob0 = sb.tile([128, 2048], bf16, tag="ob0")
            nc.scalar.activation(out=ob0, in_=acc0, func=AF.Copy)
            nc.sync.dma_start(out=o_d.ap()[0], in_=ob0)

            # --- phase 1: q[2048,4096) in freed score banks ---
            acc1a = scp.tile([128, 1024], f32, tag="slot")
            acc1b = scp.tile([128, 1024], f32, tag="slot")
            for half, acc1 in ((0, acc1a), (1, acc1b)):
                for cq in range(2):
                    qg = 2048 + half * 1024 + cq * 512
                    for p in range(8):
                        ke, ko = 2 * p, 2 * p + 1
                        me = nc.tensor.matmul(
                            acc1[0:64, cq * 512:(cq + 1) * 512],
                            lhsT=gv_all[:, ke, :],
                            rhs=e_all[:, ke * L + qg:ke * L + qg + 512],
                            tile_position=(0, 0),
                            start=(p == 0), stop=(p == 7),
                            skip_group_check=True,
                        )
                        mo = nc.tensor.matmul(
                            acc1[64:128, cq * 512:(cq + 1) * 512],
                            lhsT=gv_all[:, ko, :],
                            rhs=e_all[:, ko * L + qg:ko * L + qg + 512],
                            tile_position=(0, 64),
                            start=(p == 0), stop=(p == 7),
                            skip_group_check=True,
                        )
                        _add_dep_helper(mo.ins, me.ins, sync=False,
                                        reason="av1 pair order")
                ob1 = sb.tile([128, 1024], bf16, tag=f"ob1{half}")
                nc.scalar.activation(out=ob1, in_=acc1, func=AF.Copy)
                nc.sync.dma_start(
                    out=o_d.ap()[1][:, half * 1024:(half + 1) * 1024],
                    in_=ob1,
                )

    nc.compile()
    return nc


def _get_nc():
    if "nc" not in _cache:
        _cache["nc"] = _build()
    return _cache["nc"]


def _in_maps(x, Wq, bq, Wk, bk, Wv, bv):
    s = np.float32(AEXP / np.sqrt(np.float32(C)))
    wq1 = (np.concatenate([Wq, bq[None, :]], 0) * s).astype(np.float32)
    wq1 = np.concatenate([wq1, wq1], 1)          # doubled -> replicated QT
    wk1 = np.concatenate([Wk, bk[None, :]], 0).astype(np.float32)
    wk1 = np.concatenate([wk1, wk1], 1)
    wv1 = np.concatenate([Wv, bv[None, :]], 0).astype(np.float32)
    maps = []
    for core in range(NCORES):
        b, half = core // 2, core % 2
        x1t = np.ascontiguousarray(np.concatenate(
            [x[b], np.ones((L, 1), np.float32)], 1
        ).T.astype(np.float32))              # [65, L]
        xk = np.ascontiguousarray(x1t[:, half * KSH:(half + 1) * KSH])
        maps.append({
            "xt": x1t,
            "xk": xk,
            "wq": wq1, "wk": wk1, "wv": wv1,
        })
    return maps


def _assemble(outs, x):
    full = np.empty((B, L, C), np.float32)
    for b in range(B):
        o0, o1 = outs[2 * b], outs[2 * b + 1]
        att = (o0[0, 0:64] + o0[0, 64:128] + o1[0, 0:64] + o1[0, 64:128],
               o0[1, 0:64] + o0[1, 64:128] + o1[1, 0:64] + o1[1, 64:128])
        full[b] = np.concatenate(att, axis=1).T + x[b]
    return full


def _run(x, Wq, bq, Wk, bk, Wv, bv, trace=False):
    from concourse.bass_utils import run_bass_kernel_spmd

    nc = _get_nc()
    maps = _in_maps(x, Wq, bq, Wk, bk, Wv, bv)
    res = run_bass_kernel_spmd(
        nc, maps, core_ids=list(range(NCORES)), trace=trace
    )
    outs = [r["o"].astype(np.float32) for r in res.results]
    return _assemble(outs, x), res


def kernel(x, Wq, bq, Wk, bk, Wv, bv):
    x = np.asarray(x, np.float32)
    full, _ = _run(
        x,
        np.asarray(Wq, np.float32), np.asarray(bq, np.float32),
        np.asarray(Wk, np.float32), np.asarray(bk, np.float32),
        np.asarray(Wv, np.float32), np.asarray(bv, np.float32),
    )
    return full


# revision 19
# speedup vs baseline: 1.3017x; 1.1619x over previous
"""Trainium2 Bass kernel for nn_Attention1D (B=4, L=4096, C=64).

reference:
    Q = x@Wq + bq ; K = x@Wk + bk ; V = x@Wv + bv          (per batch b)
    s = Q @ K.T / sqrt(C)                                   [L_q, L_k]
    attn = softmax(s, axis=q)      # normalize over QUERY axis
    out = attn @ V + x

Sharding: 8 cores = 4 batches x 2 key-shards (k in [0,2048) / [2048,4096)).
softmax normalizes over q (not sharded) -> per-core softmax fully local:
    Z[k]   = sum_q exp(s[q,k]);  out_qf = sum_k exp(s[q,k]) * (V[k,f]/Z[k])
k-shards' partial outputs ADD on the host (+ residual x).

Design (exp-wall split across ScalarE+VectorE, PE restructured vs v1):
  - scores transposed sT[k,q], channel-major. Per k-tile (128 keys): 4 gens
    of [128,1024] PSUM slots (2-buf ring, 4 banks), each filled by a
    row-packed MM pair (two 512-q chunks concurrently in PE rows 0-63/64-127
    via doubled Q/K channel copies, QT/KT in bf16: per-k score offsets cancel
    in the softmax-over-q ratio, measured 5e-4 total).
  - exp split per k-tile: ScalarE ACT-Exp on q[0, 2048+SS) (free Z via
    accum_out); VectorE does q[2048+SS, 4096) with the Schraudolph int16
    bit-trick: i16 = round(s*A + B) bitcast as bf16 == exp(s)*(1+-4%); its Z
    via one tensor_reduce. A = 128/ln2 is folded into Wq host-side, the ACT
    uses scale=ln2/128 to undo it. Trick error washes out in the softmax
    ratio + 4096-key sum + residual.
  - AV: outT[f,q] PSUM-accumulated with gv = V/Z STATIONARY (64-col
    LDWEIGHTS): col-packed pairs (even k-tile -> PE cols 0-63 -> acc rows
    0:64, odd -> 64:128; per-region start=True). acc0 [128,2048] (4 banks)
    covers q[0,2048) in-loop (AV deferred behind gv); q[2048,4096) after the
    loop in the freed score banks.
  - HEAT dummy matmuls per gen keep the PE HAM at K=8/8 (cleared by the real
    MM's start=True); gv on GPSIMD; output staged bf16, host does
    out.T = o[h][0:64]+o[h][64:128], + partner core + residual.
"""

import numpy as np
import ml_dtypes  # noqa: F401

B, L, C = 4, 4096, 64
NCORES = 8
KSH = L // 2          # keys per core: 2048
NKT = KSH // 128      # 16 k-tiles per core
SS = 640              # ScalarE's share of gen2's 1024 cols (tunable)
HEAT = 0              # heater MMs per score gen (HAM K=8/8 keepalive)
GV_GPSIMD = True      # compute gv = V*rz on GPSIMD (else VectorE)
AEXP = 128.0 / np.log(2.0)          # folded into Wq
BOFF = 16256.0 - 7.42               # int16 exp bias (round-to-nearest HW)

_cache = {}


def _build():
    import concourse.bacc as bacc
    import concourse.mybir as mybir
    import concourse.tile as tile
    from concourse.bass import _add_dep_helper

    bf16 = mybir.dt.bfloat16
    f32 = mybir.dt.float32
    f32r = mybir.dt.float32r
    i16 = mybir.dt.int16
    AF = mybir.ActivationFunctionType
    ALU = mybir.AluOpType
    AX = mybir.AxisListType

    nc = bacc.Bacc("TRN2", target_bir_lowering=False, debug=False)

    xt_d = nc.dram_tensor("xt", [C + 1, L], f32r, kind="ExternalInput")
    xk_d = nc.dram_tensor("xk", [C + 1, KSH], f32r, kind="ExternalInput")
    wq_d = nc.dram_tensor("wq", [C + 1, 2 * C], f32r, kind="ExternalInput")
    wk_d = nc.dram_tensor("wk", [C + 1, 2 * C], f32r, kind="ExternalInput")
    wv_d = nc.dram_tensor("wv", [C + 1, C], f32r, kind="ExternalInput")
    o_d = nc.dram_tensor("o", [4, 128, 1024], bf16, kind="ExternalOutput")

    with tile.TileContext(nc) as tc:
        with (
            tc.tile_pool(name="consts", bufs=1) as consts,
            tc.tile_pool(name="sb", bufs=1) as sb,
            tc.tile_pool(name="scp", bufs=3, space="PSUM") as scp,
            tc.tile_pool(name="accp", bufs=1, space="PSUM") as accp,
        ):
            acc0 = accp.tile([128, 1024], f32, tag="acc")   # 2 banks

            # --- input DMAs first (sync + gpsimd queues in parallel) ---
            wq_s = consts.tile([C + 1, 2 * C], f32r)
            wk_s = consts.tile([C + 1, 2 * C], f32r)
            wv_s = consts.tile([C + 1, C], f32r)
            nc.sync.dma_start(out=wq_s, in_=wq_d.ap())
            nc.gpsimd.dma_start(out=wk_s, in_=wk_d.ap())
            nc.gpsimd.dma_start(out=wv_s, in_=wv_d.ap())

            xt_c = []
            for c in range(8):
                t = sb.tile([C + 1, 512], f32r, tag=f"xt{c}")
                eng = nc.sync if c % 2 == 0 else nc.gpsimd
                eng.dma_start(out=t, in_=xt_d.ap()[:, c * 512:(c + 1) * 512])
                xt_c.append(t)
            xk_c = []
            for c in range(4):
                t = sb.tile([C + 1, 512], f32r, tag=f"xk{c}")
                eng = nc.sync if c % 2 == 0 else nc.gpsimd
                eng.dma_start(out=t, in_=xk_d.ap()[:, c * 512:(c + 1) * 512])
                xk_c.append(t)

            # --- HAM warmup: dummy matmuls through the slot ring ---
            wu = consts.tile([128, 512], bf16)
            nc.vector.memset(wu, 0.0)
            for _ in range(12):
                wps = scp.tile([128, 1024], f32, tag="slot")
                nc.tensor.matmul(wps[:, 0:512], lhsT=wu[:, 0:128], rhs=wu,
                                 start=True, stop=True)

            # persistent SBUF state
            qt_s = sb.tile([128, 8, 512], bf16, tag="qt")
            kt2 = sb.tile([128, 4, 512], bf16, tag="kt")   # [2C, k-half]
            v_sb = sb.tile([128, NKT, C], bf16, tag="v")
            gv_all = sb.tile([128, NKT, C], bf16, tag="gv")
            e_all = sb.tile([128, NKT * L], bf16, tag="e")
            zps = sb.tile([128, NKT, 4], f32, tag="zps")
            zz = sb.tile([128, NKT], f32, tag="zz")
            rz = sb.tile([128, NKT], f32, tag="rz")
            dume = sb.tile([128, 1], bf16, tag="dume")

            # force the exp table set before any Copy-ACT evacuations
            nc.scalar.activation(out=dume, in_=wu[:, 0:1], func=AF.Exp)

            # --- projections through the slot ring, paired 512-chunks ---
            for g in range(4):
                slot = scp.tile([128, 1024], f32, tag="slot")
                for h in range(2):
                    nc.tensor.matmul(
                        slot[:, h * 512:(h + 1) * 512], lhsT=wq_s,
                        rhs=xt_c[2 * g + h], start=True, stop=True,
                    )
                if g % 2 == 0:
                    nc.scalar.activation(out=qt_s[:, 2 * g:2 * g + 2, :],
                                         in_=slot, func=AF.Copy)
                else:
                    nc.vector.tensor_copy(out=qt_s[:, 2 * g:2 * g + 2, :],
                                          in_=slot)
            for g in range(2):
                slot = scp.tile([128, 1024], f32, tag="slot")
                for h in range(2):
                    nc.tensor.matmul(
                        slot[:, h * 512:(h + 1) * 512], lhsT=wk_s,
                        rhs=xk_c[2 * g + h], start=True, stop=True,
                    )
                if g == 0:
                    nc.scalar.activation(out=kt2[:, 0:2, :], in_=slot,
                                         func=AF.Copy)
                else:
                    nc.vector.tensor_copy(out=kt2[:, 2:4, :], in_=slot)
            # V: one gen holds all 16 k-tiles' [128,64]
            vslot = scp.tile([128, 1024], f32, tag="slot")
            for kt in range(NKT):
                nc.tensor.matmul(
                    vslot[:, kt * C:(kt + 1) * C],
                    lhsT=xk_c[kt // 4][:, (kt % 4) * 128:(kt % 4 + 1) * 128],
                    rhs=wv_s, start=True, stop=True,
                )
            nc.vector.tensor_copy(out=v_sb, in_=vslot)

            def kslice(kt, r0, r1):
                return kt2[r0:r1, kt // 4, (kt % 4) * 128:(kt % 4 + 1) * 128]

            # --- main loop over k-pairs ---
            av_queue = []   # deferred phase-0 AV emitters

            def emit_av_pair(p):
                # phase-0 AV for pair p: q[0,1024) in 2 chunks, col-packed
                ke, ko = 2 * p, 2 * p + 1
                ops = []
                for cq in range(2):
                    def mk(cq=cq, ke=ke, ko=ko, p=p):
                        me = nc.tensor.matmul(
                            acc0[0:64, cq * 512:(cq + 1) * 512],
                            lhsT=gv_all[:, ke, :],
                            rhs=e_all[:, ke * L + cq * 512:ke * L + (cq + 1) * 512],
                            tile_position=(0, 0),
                            start=(p == 0), stop=(p == 7),
                            skip_group_check=True,
                        )
                        mo = nc.tensor.matmul(
                            acc0[64:128, cq * 512:(cq + 1) * 512],
                            lhsT=gv_all[:, ko, :],
                            rhs=e_all[:, ko * L + cq * 512:ko * L + (cq + 1) * 512],
                            tile_position=(0, 64),
                            start=(p == 0), stop=(p == 7),
                            skip_group_check=True,
                        )
                        _add_dep_helper(mo.ins, me.ins, sync=False,
                                        reason="av pair order")
                    ops.append(mk)
                return ops

            def drain_av(n):
                for _ in range(n):
                    if av_queue:
                        av_queue.pop(0)()

            for p in range(8):
                for kt in (2 * p, 2 * p + 1):
                    lA = kslice(kt, 0, C)
                    lB = kslice(kt, C, 128)
                    for g in range(4):
                        slot = scp.tile([128, 1024], f32, tag="slot")
                        # heaters keep HAM busy; cleared by real start=True
                        for _ in range(HEAT):
                            nc.tensor.matmul(slot[:, 0:512],
                                             lhsT=wu[:, 0:128], rhs=wu,
                                             start=True, stop=True)
                        qc = 2 * g
                        ma = nc.tensor.matmul(
                            slot[:, 0:512], lhsT=lA,
                            rhs=qt_s[0:C, qc, :], tile_position=(0, 0),
                            start=True, stop=True,
                        )
                        mb = nc.tensor.matmul(
                            slot[:, 512:1024], lhsT=lB,
                            rhs=qt_s[C:128, qc + 1, :], tile_position=(C, 0),
                            start=True, stop=True,
                        )
                        _add_dep_helper(mb.ins, ma.ins, sync=False,
                                        reason="score pair order")
                        q0 = g * 1024
                        if g < 2:
                            nc.scalar.activation(
                                out=e_all[:, kt * L + q0:kt * L + q0 + 1024], in_=slot,
                                func=AF.Exp, scale=float(np.log(2.0) / 128.0),
                                accum_out=zps[:, kt, g:g + 1],
                            )
                        elif g == 2:
                            nc.scalar.activation(
                                out=e_all[:, kt * L + q0:kt * L + q0 + SS], in_=slot[:, 0:SS],
                                func=AF.Exp, scale=float(np.log(2.0) / 128.0),
                                accum_out=zps[:, kt, 2:3],
                            )
                            nc.vector.tensor_scalar(
                                out=e_all[:, kt * L + q0 + SS:kt * L + q0 + 1024].bitcast(i16),
                                in0=slot[:, SS:1024], scalar1=BOFF,
                                scalar2=None, op0=ALU.add,
                            )
                        else:
                            nc.vector.tensor_scalar(
                                out=e_all[:, kt * L + q0:kt * L + q0 + 1024].bitcast(i16),
                                in0=slot, scalar1=BOFF,
                                scalar2=None, op0=ALU.add,
                            )
                        drain_av(1)
                    # Z for the DVE range
                    nc.vector.tensor_reduce(
                        out=zps[:, kt, 3:4], in_=e_all[:, kt * L + 2048 + SS:(kt + 1) * L],
                        axis=AX.X, op=ALU.add,
                    )
                if p % 2 == 1:
                    # z-combine + reciprocal + gv for k-tiles 4*(p//2)..+4
                    j = 4 * (p // 2)
                    nc.vector.tensor_reduce(
                        out=zz[:, j:j + 4], in_=zps[:, j:j + 4, :],
                        axis=AX.X, op=ALU.add,
                    )
                    nc.vector.reciprocal(out=rz[:, j:j + 4], in_=zz[:, j:j + 4])
                    for kt in range(j, j + 4):
                        if GV_GPSIMD:
                            nc.gpsimd.tensor_scalar(
                                out=gv_all[:, kt, :], in0=v_sb[:, kt, :],
                                scalar1=rz[:, kt:kt + 1], scalar2=None,
                                op0=ALU.mult,
                            )
                        else:
                            nc.vector.tensor_scalar_mul(
                                gv_all[:, kt, :], v_sb[:, kt, :],
                                rz[:, kt:kt + 1]
                            )
                    # their gv is ready: enqueue AV for pairs p-1, p
                    av_queue.extend(emit_av_pair(p - 1))
                    av_queue.extend(emit_av_pair(p))

            # drain remaining phase-0 AV (pairs 6,7)
            drain_av(len(av_queue))

            ob0 = sb.tile([128, 1024], bf16, tag="ob0")
            nc.scalar.activation(out=ob0, in_=acc0, func=AF.Copy)
            nc.sync.dma_start(out=o_d.ap()[0], in_=ob0)

            # --- phases 1-3: q[1024,4096) as a dense PE stream in the ring ---
            for ph in range(1, 4):
                acc1 = scp.tile([128, 1024], f32, tag="slot")
                for cq in range(2):
                    qg = ph * 1024 + cq * 512
                    for p in range(8):
                        ke, ko = 2 * p, 2 * p + 1
                        me = nc.tensor.matmul(
                            acc1[0:64, cq * 512:(cq + 1) * 512],
                            lhsT=gv_all[:, ke, :],
                            rhs=e_all[:, ke * L + qg:ke * L + qg + 512],
                            tile_position=(0, 0),
                            start=(p == 0), stop=(p == 7),
                            skip_group_check=True,
                        )
                        mo = nc.tensor.matmul(
                            acc1[64:128, cq * 512:(cq + 1) * 512],
                            lhsT=gv_all[:, ko, :],
                            rhs=e_all[:, ko * L + qg:ko * L + qg + 512],
                            tile_position=(0, 64),
                            start=(p == 0), stop=(p == 7),
                            skip_group_check=True,
                        )
                        _add_dep_helper(mo.ins, me.ins, sync=False,
                                        reason="av1 pair order")
                ob1 = sb.tile([128, 1024], bf16, tag=f"ob1{ph}")
                nc.scalar.activation(out=ob1, in_=acc1, func=AF.Copy)
                nc.sync.dma_start(out=o_d.ap()[ph], in_=ob1)

    nc.compile()
    return nc


def _get_nc():
    if "nc" not in _cache:
        _cache["nc"] = _build()
    return _cache["nc"]


def _in_maps(x, Wq, bq, Wk, bk, Wv, bv):
    s = np.float32(AEXP / np.sqrt(np.float32(C)))
    wq1 = (np.concatenate([Wq, bq[None, :]], 0) * s).astype(np.float32)
    wq1 = np.concatenate([wq1, wq1], 1)          # doubled -> replicated QT
    wk1 = np.concatenate([Wk, bk[None, :]], 0).astype(np.float32)
    wk1 = np.concatenate([wk1, wk1], 1)
    wv1 = np.concatenate([Wv, bv[None, :]], 0).astype(np.float32)
    maps = []
    for core in range(NCORES):
        b, half = core // 2, core % 2
        x1t = np.ascontiguousarray(np.concatenate(
            [x[b], np.ones((L, 1), np.float32)], 1
        ).T.astype(np.float32))              # [65, L]
        xk = np.ascontiguousarray(x1t[:, half * KSH:(half + 1) * KSH])
        maps.append({
            "xt": x1t,
            "xk": xk,
            "wq": wq1, "wk": wk1, "wv": wv1,
        })
    return maps


def _assemble(outs, x):
    full = np.empty((B, L, C), np.float32)
    for b in range(B):
        o0, o1 = outs[2 * b], outs[2 * b + 1]
        att = tuple(
            o0[ph, 0:64] + o0[ph, 64:128] + o1[ph, 0:64] + o1[ph, 64:128]
            for ph in range(4)
        )
        full[b] = np.concatenate(att, axis=1).T + x[b]
    return full


def _run(x, Wq, bq, Wk, bk, Wv, bv, trace=False):
    from concourse.bass_utils import run_bass_kernel_spmd

    nc = _get_nc()
    maps = _in_maps(x, Wq, bq, Wk, bk, Wv, bv)
    res = run_bass_kernel_spmd(
        nc, maps, core_ids=list(range(NCORES)), trace=trace
    )
    outs = [r["o"].astype(np.float32) for r in res.results]
    return _assemble(outs, x), res


def kernel(x, Wq, bq, Wk, bk, Wv, bv):
    x = np.asarray(x, np.float32)
    full, _ = _run(
        x,
        np.asarray(Wq, np.float32), np.asarray(bq, np.float32),
        np.asarray(Wk, np.float32), np.asarray(bk, np.float32),
        np.asarray(Wv, np.float32), np.asarray(bv, np.float32),
    )
    return full


# revision 20
# speedup vs baseline: 1.3813x; 1.0611x over previous
"""Trainium2 Bass kernel for nn_Attention1D (B=4, L=4096, C=64).

reference:
    Q = x@Wq + bq ; K = x@Wk + bk ; V = x@Wv + bv          (per batch b)
    s = Q @ K.T / sqrt(C)                                   [L_q, L_k]
    attn = softmax(s, axis=q)      # normalize over QUERY axis
    out = attn @ V + x

Sharding: 8 cores = 4 batches x 2 key-shards (k in [0,2048) / [2048,4096)).
softmax normalizes over q (not sharded) -> per-core softmax fully local:
    Z[k]   = sum_q exp(s[q,k]);  out_qf = sum_k exp(s[q,k]) * (V[k,f]/Z[k])
k-shards' partial outputs ADD on the host (+ residual x).

Design (exp-wall split across ScalarE+VectorE, PE restructured vs v1):
  - scores transposed sT[k,q], channel-major. Per k-tile (128 keys): 4 gens
    of [128,1024] PSUM slots (2-buf ring, 4 banks), each filled by a
    row-packed MM pair (two 512-q chunks concurrently in PE rows 0-63/64-127
    via doubled Q/K channel copies, QT/KT in bf16: per-k score offsets cancel
    in the softmax-over-q ratio, measured 5e-4 total).
  - exp split per k-tile: ScalarE ACT-Exp on q[0, 2048+SS) (free Z via
    accum_out); VectorE does q[2048+SS, 4096) with the Schraudolph int16
    bit-trick: i16 = round(s*A + B) bitcast as bf16 == exp(s)*(1+-4%); its Z
    via one tensor_reduce. A = 128/ln2 is folded into Wq host-side, the ACT
    uses scale=ln2/128 to undo it. Trick error washes out in the softmax
    ratio + 4096-key sum + residual.
  - AV: outT[f,q] PSUM-accumulated with gv = V/Z STATIONARY (64-col
    LDWEIGHTS): col-packed pairs (even k-tile -> PE cols 0-63 -> acc rows
    0:64, odd -> 64:128; per-region start=True). acc0 [128,2048] (4 banks)
    covers q[0,2048) in-loop (AV deferred behind gv); q[2048,4096) after the
    loop in the freed score banks.
  - HEAT dummy matmuls per gen keep the PE HAM at K=8/8 (cleared by the real
    MM's start=True); gv on GPSIMD; output staged bf16, host does
    out.T = o[h][0:64]+o[h][64:128], + partner core + residual.
"""

import numpy as np
import ml_dtypes  # noqa: F401

B, L, C = 4, 4096, 64
NCORES = 8
KSH = L // 2          # keys per core: 2048
NKT = KSH // 128      # 16 k-tiles per core
SS = 768              # ScalarE's share of gen2's 1024 cols (tunable)
HEAT = 0              # heater MMs per score gen (HAM K=8/8 keepalive)
GV_GPSIMD = True      # compute gv = V*rz on GPSIMD (else VectorE)
AEXP = 128.0 / np.log(2.0)          # folded into Wq
BOFF = 16256.0 - 7.42               # int16 exp bias (round-to-nearest HW)

_cache = {}


def _build():
    import concourse.bacc as bacc
    import concourse.mybir as mybir
    import concourse.tile as tile
    from concourse.bass import _add_dep_helper

    bf16 = mybir.dt.bfloat16
    f32 = mybir.dt.float32
    f32r = mybir.dt.float32r
    i16 = mybir.dt.int16
    AF = mybir.ActivationFunctionType
    ALU = mybir.AluOpType
    AX = mybir.AxisListType

    nc = bacc.Bacc("TRN2", target_bir_lowering=False, debug=False)

    xt_d = nc.dram_tensor("xt", [C + 1, L], f32r, kind="ExternalInput")
    xk_d = nc.dram_tensor("xk", [C + 1, KSH], f32r, kind="ExternalInput")
    wq_d = nc.dram_tensor("wq", [C + 1, 2 * C], f32r, kind="ExternalInput")
    wk_d = nc.dram_tensor("wk", [C + 1, 2 * C], f32r, kind="ExternalInput")
    wv_d = nc.dram_tensor("wv", [C + 1, C], f32r, kind="ExternalInput")
    o_d = nc.dram_tensor("o", [4, 128, 1024], bf16, kind="ExternalOutput")

    with tile.TileContext(nc) as tc:
        with (
            tc.tile_pool(name="consts", bufs=1) as consts,
            tc.tile_pool(name="sb", bufs=1) as sb,
            tc.tile_pool(name="scp", bufs=1, space="PSUM") as scp,
        ):
            # ONE PSUM tile: 4 x [128,1024] regions (8 banks), subtile-dep ring
            sc4 = scp.tile([128, 4, 1024], f32, tag="sc4")

            # --- input DMAs first (sync + gpsimd queues in parallel) ---
            wq_s = consts.tile([C + 1, 2 * C], f32r)
            wk_s = consts.tile([C + 1, 2 * C], f32r)
            wv_s = consts.tile([C + 1, C], f32r)
            nc.sync.dma_start(out=wq_s, in_=wq_d.ap())
            nc.gpsimd.dma_start(out=wk_s, in_=wk_d.ap())
            nc.gpsimd.dma_start(out=wv_s, in_=wv_d.ap())

            xt_c = []
            for c in range(8):
                t = sb.tile([C + 1, 512], f32r, tag=f"xt{c}")
                eng = nc.sync if c % 2 == 0 else nc.gpsimd
                eng.dma_start(out=t, in_=xt_d.ap()[:, c * 512:(c + 1) * 512])
                xt_c.append(t)
            xk_c = []
            for c in range(4):
                t = sb.tile([C + 1, 512], f32r, tag=f"xk{c}")
                eng = nc.sync if c % 2 == 0 else nc.gpsimd
                eng.dma_start(out=t, in_=xk_d.ap()[:, c * 512:(c + 1) * 512])
                xk_c.append(t)

            # --- HAM warmup while DMAs stream ---
            wu = consts.tile([128, 512], bf16)
            nc.vector.memset(wu, 0.0)
            for _ in range(12):
                nc.tensor.matmul(sc4[:, 0, 0:512], lhsT=wu[:, 0:128], rhs=wu,
                                 start=True, stop=True)

            # persistent SBUF state
            qt_s = sb.tile([128, 8, 512], bf16, tag="qt")
            kt2 = sb.tile([128, 4, 512], bf16, tag="kt")   # [2C, k-half]
            v_sb = sb.tile([128, NKT, C], bf16, tag="v")
            gv_all = sb.tile([128, NKT, C], bf16, tag="gv")
            e_all = sb.tile([128, NKT * L], bf16, tag="e")
            zps = sb.tile([128, NKT, 3], f32, tag="zps")
            zz = sb.tile([128, NKT], f32, tag="zz")
            rz = sb.tile([128, NKT], f32, tag="rz")
            dume = sb.tile([128, 1], bf16, tag="dume")

            # force the exp table set before any Copy-ACT evacuations
            nc.scalar.activation(out=dume, in_=wu[:, 0:1], func=AF.Exp)

            # --- projections: QT fills all 4 regions, ONE evac ACT ---
            for h in range(8):
                nc.tensor.matmul(
                    sc4[:, h // 2, (h % 2) * 512:(h % 2 + 1) * 512],
                    lhsT=wq_s, rhs=xt_c[h], start=True, stop=True,
                )
            nc.scalar.activation(out=qt_s, in_=sc4, func=AF.Copy)
            # KT -> regions 0,1 ; V -> region 2
            for h in range(4):
                nc.tensor.matmul(
                    sc4[:, h // 2, (h % 2) * 512:(h % 2 + 1) * 512],
                    lhsT=wk_s, rhs=xk_c[h], start=True, stop=True,
                )
            for kt in range(NKT):
                nc.tensor.matmul(
                    sc4[:, 2, kt * C:(kt + 1) * C],
                    lhsT=xk_c[kt // 4][:, (kt % 4) * 128:(kt % 4 + 1) * 128],
                    rhs=wv_s, start=True, stop=True,
                )
            nc.vector.tensor_copy(out=kt2, in_=sc4[:, 0:2, :])
            nc.vector.tensor_copy(out=v_sb, in_=sc4[:, 2, :])

            def kslice(kt, r0, r1):
                return kt2[r0:r1, kt // 4, (kt % 4) * 128:(kt % 4 + 1) * 128]

            # --- main loop: scores + exp only (AV all post-loop) ---
            for p in range(8):
                for kt in (2 * p, 2 * p + 1):
                    lA = kslice(kt, 0, C)
                    lB = kslice(kt, C, 128)
                    for g in range(4):
                        qc = 2 * g
                        ma = nc.tensor.matmul(
                            sc4[:, g, 0:512], lhsT=lA,
                            rhs=qt_s[0:C, qc, :], tile_position=(0, 0),
                            start=True, stop=True,
                        )
                        mb = nc.tensor.matmul(
                            sc4[:, g, 512:1024], lhsT=lB,
                            rhs=qt_s[C:128, qc + 1, :], tile_position=(C, 0),
                            start=True, stop=True,
                        )
                        _add_dep_helper(mb.ins, ma.ins, sync=False,
                                        reason="score pair order")
                    # one big ACT for q[0,2048), one partial for gen2
                    nc.scalar.activation(
                        out=e_all[:, kt * L:kt * L + 2048], in_=sc4[:, 0:2, :],
                        func=AF.Exp, scale=float(np.log(2.0) / 128.0),
                        accum_out=zps[:, kt, 0:1],
                    )
                    nc.scalar.activation(
                        out=e_all[:, kt * L + 2048:kt * L + 2048 + SS],
                        in_=sc4[:, 2, 0:SS],
                        func=AF.Exp, scale=float(np.log(2.0) / 128.0),
                        accum_out=zps[:, kt, 1:2],
                    )
                    nc.vector.tensor_scalar(
                        out=e_all[:, kt * L + 2048 + SS:kt * L + 3072].bitcast(i16),
                        in0=sc4[:, 2, SS:1024], scalar1=BOFF,
                        scalar2=None, op0=ALU.add,
                    )
                    nc.vector.tensor_scalar(
                        out=e_all[:, kt * L + 3072:(kt + 1) * L].bitcast(i16),
                        in0=sc4[:, 3, :], scalar1=BOFF,
                        scalar2=None, op0=ALU.add,
                    )
                    nc.vector.tensor_reduce(
                        out=zps[:, kt, 2:3],
                        in_=e_all[:, kt * L + 2048 + SS:(kt + 1) * L],
                        axis=AX.X, op=ALU.add,
                    )
                if p % 2 == 1:
                    j = 4 * (p // 2)
                    nc.vector.tensor_reduce(
                        out=zz[:, j:j + 4], in_=zps[:, j:j + 4, :],
                        axis=AX.X, op=ALU.add,
                    )
                    nc.vector.reciprocal(out=rz[:, j:j + 4], in_=zz[:, j:j + 4])
                    for kt in range(j, j + 4):
                        if GV_GPSIMD:
                            nc.gpsimd.tensor_scalar(
                                out=gv_all[:, kt, :], in0=v_sb[:, kt, :],
                                scalar1=rz[:, kt:kt + 1], scalar2=None,
                                op0=ALU.mult,
                            )
                        else:
                            nc.vector.tensor_scalar_mul(
                                gv_all[:, kt, :], v_sb[:, kt, :],
                                rz[:, kt:kt + 1]
                            )

            # --- AV: dense post-loop stream, 4 phase regions in sc4 ---
            for ph in range(4):
                for cq in range(2):
                    qg = ph * 1024 + cq * 512
                    for p in range(8):
                        ke, ko = 2 * p, 2 * p + 1
                        me = nc.tensor.matmul(
                            sc4[0:64, ph, cq * 512:(cq + 1) * 512],
                            lhsT=gv_all[:, ke, :],
                            rhs=e_all[:, ke * L + qg:ke * L + qg + 512],
                            tile_position=(0, 0),
                            start=(p == 0), stop=(p == 7),
                            skip_group_check=True,
                        )
                        mo = nc.tensor.matmul(
                            sc4[64:128, ph, cq * 512:(cq + 1) * 512],
                            lhsT=gv_all[:, ko, :],
                            rhs=e_all[:, ko * L + qg:ko * L + qg + 512],
                            tile_position=(0, 64),
                            start=(p == 0), stop=(p == 7),
                            skip_group_check=True,
                        )
                        _add_dep_helper(mo.ins, me.ins, sync=False,
                                        reason="av pair order")
                ob1 = sb.tile([128, 1024], bf16, tag=f"ob{ph}")
                nc.scalar.activation(out=ob1, in_=sc4[:, ph, :], func=AF.Copy)
                nc.sync.dma_start(out=o_d.ap()[ph], in_=ob1)

    nc.compile()
    return nc


def _get_nc():
    if "nc" not in _cache:
        _cache["nc"] = _build()
    return _cache["nc"]


def _in_maps(x, Wq, bq, Wk, bk, Wv, bv):
    s = np.float32(AEXP / np.sqrt(np.float32(C)))
    wq1 = (np.concatenate([Wq, bq[None, :]], 0) * s).astype(np.float32)
    wq1 = np.concatenate([wq1, wq1], 1)          # doubled -> replicated QT
    wk1 = np.concatenate([Wk, bk[None, :]], 0).astype(np.float32)
    wk1 = np.concatenate([wk1, wk1], 1)
    wv1 = np.concatenate([Wv, bv[None, :]], 0).astype(np.float32)
    maps = []
    for core in range(NCORES):
        b, half = core // 2, core % 2
        x1t = np.ascontiguousarray(np.concatenate(
            [x[b], np.ones((L, 1), np.float32)], 1
        ).T.astype(np.float32))              # [65, L]
        xk = np.ascontiguousarray(x1t[:, half * KSH:(half + 1) * KSH])
        maps.append({
            "xt": x1t,
            "xk": xk,
            "wq": wq1, "wk": wk1, "wv": wv1,
        })
    return maps


def _assemble(outs, x):
    full = np.empty((B, L, C), np.float32)
    for b in range(B):
        o0, o1 = outs[2 * b], outs[2 * b + 1]
        att = tuple(
            o0[ph, 0:64] + o0[ph, 64:128] + o1[ph, 0:64] + o1[ph, 64:128]
            for ph in range(4)
        )
        full[b] = np.concatenate(att, axis=1).T + x[b]
    return full


def _run(x, Wq, bq, Wk, bk, Wv, bv, trace=False):
    from concourse.bass_utils import run_bass_kernel_spmd

    nc = _get_nc()
    maps = _in_maps(x, Wq, bq, Wk, bk, Wv, bv)
    res = run_bass_kernel_spmd(
        nc, maps, core_ids=list(range(NCORES)), trace=trace
    )
    outs = [r["o"].astype(np.float32) for r in res.results]
    return _assemble(outs, x), res


def kernel(x, Wq, bq, Wk, bk, Wv, bv):
    x = np.asarray(x, np.float32)
    full, _ = _run(
        x,
        np.asarray(Wq, np.float32), np.asarray(bq, np.float32),
        np.asarray(Wk, np.float32), np.asarray(bk, np.float32),
        np.asarray(Wv, np.float32), np.asarray(bv, np.float32),
    )
    return full


# revision 21
# speedup vs baseline: 1.5040x; 1.0888x over previous
"""Trainium2 Bass kernel for nn_Attention1D (B=4, L=4096, C=64).

reference:
    Q = x@Wq + bq ; K = x@Wk + bk ; V = x@Wv + bv          (per batch b)
    s = Q @ K.T / sqrt(C)                                   [L_q, L_k]
    attn = softmax(s, axis=q)      # normalize over QUERY axis
    out = attn @ V + x

Sharding: 8 cores = 4 batches x 2 key-shards (k in [0,2048) / [2048,4096)).
softmax normalizes over q (not sharded) -> per-core softmax fully local:
    Z[k]   = sum_q exp(s[q,k]);  out_qf = sum_k exp(s[q,k]) * (V[k,f]/Z[k])
k-shards' partial outputs ADD on the host (+ residual x).

Design (exp-wall split across ScalarE+VectorE, PE restructured vs v1):
  - scores transposed sT[k,q], channel-major. Per k-tile (128 keys): 4 gens
    of [128,1024] PSUM slots (2-buf ring, 4 banks), each filled by a
    row-packed MM pair (two 512-q chunks concurrently in PE rows 0-63/64-127
    via doubled Q/K channel copies, QT/KT in bf16: per-k score offsets cancel
    in the softmax-over-q ratio, measured 5e-4 total).
  - exp split per k-tile: ScalarE ACT-Exp on q[0, 2048+SS) (free Z via
    accum_out); VectorE does q[2048+SS, 4096) with the Schraudolph int16
    bit-trick: i16 = round(s*A + B) bitcast as bf16 == exp(s)*(1+-4%); its Z
    via one tensor_reduce. A = 128/ln2 is folded into Wq host-side, the ACT
    uses scale=ln2/128 to undo it. Trick error washes out in the softmax
    ratio + 4096-key sum + residual.
  - AV: outT[f,q] PSUM-accumulated with gv = V/Z STATIONARY (64-col
    LDWEIGHTS): col-packed pairs (even k-tile -> PE cols 0-63 -> acc rows
    0:64, odd -> 64:128; per-region start=True). acc0 [128,2048] (4 banks)
    covers q[0,2048) in-loop (AV deferred behind gv); q[2048,4096) after the
    loop in the freed score banks.
  - HEAT dummy matmuls per gen keep the PE HAM at K=8/8 (cleared by the real
    MM's start=True); gv on GPSIMD; output staged bf16, host does
    out.T = o[h][0:64]+o[h][64:128], + partner core + residual.
"""

import numpy as np
import ml_dtypes  # noqa: F401

B, L, C = 4, 4096, 64
NCORES = 8
KSH = L // 2          # keys per core: 2048
NKT = KSH // 128      # 16 k-tiles per core
SS = 768              # ScalarE's share of gen2's 1024 cols (tunable)
HEAT = 0              # heater MMs per score gen (HAM K=8/8 keepalive)
GV_GPSIMD = True      # compute gv = V*rz on GPSIMD (else VectorE)
AEXP = 128.0 / np.log(2.0)          # folded into Wq
BOFF = 16256.0 - 7.42               # int16 exp bias (round-to-nearest HW)

_cache = {}


def _build():
    import concourse.bacc as bacc
    import concourse.mybir as mybir
    import concourse.tile as tile
    from concourse.bass import _add_dep_helper

    bf16 = mybir.dt.bfloat16
    f32 = mybir.dt.float32
    f32r = mybir.dt.float32r
    i16 = mybir.dt.int16
    AF = mybir.ActivationFunctionType
    ALU = mybir.AluOpType
    AX = mybir.AxisListType

    nc = bacc.Bacc("TRN2", target_bir_lowering=False, debug=False)

    xt_d = nc.dram_tensor("xt", [C + 1, L], f32r, kind="ExternalInput")
    xk_d = nc.dram_tensor("xk", [C + 1, KSH], f32r, kind="ExternalInput")
    wq_d = nc.dram_tensor("wq", [C + 1, 2 * C], f32r, kind="ExternalInput")
    wk_d = nc.dram_tensor("wk", [C + 1, 2 * C], f32r, kind="ExternalInput")
    wv_d = nc.dram_tensor("wv", [C + 1, C], f32r, kind="ExternalInput")
    o_d = nc.dram_tensor("o", [4, 128, 1024], bf16, kind="ExternalOutput")

    with tile.TileContext(nc) as tc:
        with (
            tc.tile_pool(name="consts", bufs=1) as consts,
            tc.tile_pool(name="sb", bufs=1) as sb,
            tc.tile_pool(name="scp", bufs=1, space="PSUM") as scp,
        ):
            # ONE PSUM tile: 4 x [128,1024] regions (8 banks), subtile-dep ring
            sc4 = scp.tile([128, 4, 1024], f32, tag="sc4")

            # --- input DMAs first (sync + gpsimd queues in parallel) ---
            wq_s = consts.tile([C + 1, 2 * C], f32r)
            wk_s = consts.tile([C + 1, 2 * C], f32r)
            wv_s = consts.tile([C + 1, C], f32r)
            nc.sync.dma_start(out=wq_s, in_=wq_d.ap())
            nc.gpsimd.dma_start(out=wk_s, in_=wk_d.ap())
            nc.gpsimd.dma_start(out=wv_s, in_=wv_d.ap())

            xt_c = []
            for c in range(8):
                t = sb.tile([C + 1, 512], f32r, tag=f"xt{c}")
                eng = nc.sync if c % 2 == 0 else nc.gpsimd
                eng.dma_start(out=t, in_=xt_d.ap()[:, c * 512:(c + 1) * 512])
                xt_c.append(t)
            xk_c = []
            for c in range(4):
                t = sb.tile([C + 1, 512], f32r, tag=f"xk{c}")
                eng = nc.sync if c % 2 == 0 else nc.gpsimd
                eng.dma_start(out=t, in_=xk_d.ap()[:, c * 512:(c + 1) * 512])
                xk_c.append(t)

            # --- HAM warmup while DMAs stream ---
            wu = consts.tile([128, 512], bf16)
            nc.vector.memset(wu, 0.0)
            for _ in range(12):
                nc.tensor.matmul(sc4[:, 0, 0:512], lhsT=wu[:, 0:128], rhs=wu,
                                 start=True, stop=True)

            # persistent SBUF state
            qt_s = sb.tile([128, 8, 512], bf16, tag="qt")
            kt2 = sb.tile([128, 4, 512], bf16, tag="kt")   # [2C, k-half]
            v_sb = sb.tile([128, NKT, C], bf16, tag="v")
            gv_all = sb.tile([128, NKT, C], bf16, tag="gv")
            e_all = sb.tile([128, NKT * L], bf16, tag="e")
            zps = sb.tile([128, NKT, 3], f32, tag="zps")
            zz = sb.tile([128, NKT], f32, tag="zz")
            rz = sb.tile([128, NKT], f32, tag="rz")
            dume = sb.tile([128, 1], bf16, tag="dume")

            # force the exp table set before any Copy-ACT evacuations
            nc.scalar.activation(out=dume, in_=wu[:, 0:1], func=AF.Exp)

            # --- projections: QT fills all 4 regions, ONE evac ACT ---
            for h in range(8):
                nc.tensor.matmul(
                    sc4[:, h // 2, (h % 2) * 512:(h % 2 + 1) * 512],
                    lhsT=wq_s, rhs=xt_c[h], start=True, stop=True,
                )
                if h == 3:
                    nc.scalar.activation(out=qt_s[:, 0:4, :],
                                         in_=sc4[:, 0:2, :], func=AF.Copy)
            nc.scalar.activation(out=qt_s[:, 4:8, :], in_=sc4[:, 2:4, :],
                                 func=AF.Copy)
            # KT -> regions 0,1 ; V -> region 2
            for h in range(4):
                nc.tensor.matmul(
                    sc4[:, h // 2, (h % 2) * 512:(h % 2 + 1) * 512],
                    lhsT=wk_s, rhs=xk_c[h], start=True, stop=True,
                )
            for kt in range(NKT):
                nc.tensor.matmul(
                    sc4[:, 2, kt * C:(kt + 1) * C],
                    lhsT=xk_c[kt // 4][:, (kt % 4) * 128:(kt % 4 + 1) * 128],
                    rhs=wv_s, start=True, stop=True,
                )
            nc.vector.tensor_copy(out=kt2, in_=sc4[:, 0:2, :])
            nc.vector.tensor_copy(out=v_sb, in_=sc4[:, 2, :])

            def kslice(kt, r0, r1):
                return kt2[r0:r1, kt // 4, (kt % 4) * 128:(kt % 4 + 1) * 128]

            # --- main loop: scores + exp only (AV all post-loop) ---
            for p in range(8):
                for kt in (2 * p, 2 * p + 1):
                    lA = kslice(kt, 0, C)
                    lB = kslice(kt, C, 128)
                    for g in range(4):
                        qc = 2 * g
                        ma = nc.tensor.matmul(
                            sc4[:, g, 0:512], lhsT=lA,
                            rhs=qt_s[0:C, qc, :], tile_position=(0, 0),
                            start=True, stop=True,
                        )
                        mb = nc.tensor.matmul(
                            sc4[:, g, 512:1024], lhsT=lB,
                            rhs=qt_s[C:128, qc + 1, :], tile_position=(C, 0),
                            start=True, stop=True,
                        )
                        _add_dep_helper(mb.ins, ma.ins, sync=False,
                                        reason="score pair order")
                    # one big ACT for q[0,2048), one partial for gen2
                    nc.scalar.activation(
                        out=e_all[:, kt * L:kt * L + 2048], in_=sc4[:, 0:2, :],
                        func=AF.Exp, scale=float(np.log(2.0) / 128.0),
                        accum_out=zps[:, kt, 0:1],
                    )
                    nc.scalar.activation(
                        out=e_all[:, kt * L + 2048:kt * L + 2048 + SS],
                        in_=sc4[:, 2, 0:SS],
                        func=AF.Exp, scale=float(np.log(2.0) / 128.0),
                        accum_out=zps[:, kt, 1:2],
                    )
                    nc.vector.tensor_scalar(
                        out=e_all[:, kt * L + 2048 + SS:kt * L + 3072].bitcast(i16),
                        in0=sc4[:, 2, SS:1024], scalar1=BOFF,
                        scalar2=None, op0=ALU.add,
                    )
                    nc.vector.tensor_scalar(
                        out=e_all[:, kt * L + 3072:(kt + 1) * L].bitcast(i16),
                        in0=sc4[:, 3, :], scalar1=BOFF,
                        scalar2=None, op0=ALU.add,
                    )
                    nc.vector.tensor_reduce(
                        out=zps[:, kt, 2:3],
                        in_=e_all[:, kt * L + 2048 + SS:(kt + 1) * L],
                        axis=AX.X, op=ALU.add,
                    )
                if p % 2 == 1:
                    j = 4 * (p // 2)
                    nc.vector.tensor_reduce(
                        out=zz[:, j:j + 4], in_=zps[:, j:j + 4, :],
                        axis=AX.X, op=ALU.add,
                    )
                    nc.vector.reciprocal(out=rz[:, j:j + 4], in_=zz[:, j:j + 4])
                    for kt in range(j, j + 4):
                        if GV_GPSIMD:
                            nc.gpsimd.tensor_scalar(
                                out=gv_all[:, kt, :], in0=v_sb[:, kt, :],
                                scalar1=rz[:, kt:kt + 1], scalar2=None,
                                op0=ALU.mult,
                            )
                        else:
                            nc.vector.tensor_scalar_mul(
                                gv_all[:, kt, :], v_sb[:, kt, :],
                                rz[:, kt:kt + 1]
                            )

            # --- AV: dense post-loop stream, p-major over 4 phase regions ---
            for p in range(8):
                ke, ko = 2 * p, 2 * p + 1
                for ph in range(4):
                    for cq in range(2):
                        qg = ph * 1024 + cq * 512
                        me = nc.tensor.matmul(
                            sc4[0:64, ph, cq * 512:(cq + 1) * 512],
                            lhsT=gv_all[:, ke, :],
                            rhs=e_all[:, ke * L + qg:ke * L + qg + 512],
                            tile_position=(0, 0),
                            start=(p == 0), stop=(p == 7),
                            skip_group_check=True,
                        )
                        mo = nc.tensor.matmul(
                            sc4[64:128, ph, cq * 512:(cq + 1) * 512],
                            lhsT=gv_all[:, ko, :],
                            rhs=e_all[:, ko * L + qg:ko * L + qg + 512],
                            tile_position=(0, 64),
                            start=(p == 0), stop=(p == 7),
                            skip_group_check=True,
                        )
                        _add_dep_helper(mo.ins, me.ins, sync=False,
                                        reason="av pair order")
            for ph in range(4):
                ob1 = sb.tile([128, 1024], bf16, tag=f"ob{ph}")
                nc.scalar.activation(out=ob1, in_=sc4[:, ph, :], func=AF.Copy)
                nc.sync.dma_start(out=o_d.ap()[ph], in_=ob1)

    nc.compile()
    return nc


def _get_nc():
    if "nc" not in _cache:
        _cache["nc"] = _build()
    return _cache["nc"]


def _in_maps(x, Wq, bq, Wk, bk, Wv, bv):
    s = np.float32(AEXP / np.sqrt(np.float32(C)))
    wq1 = (np.concatenate([Wq, bq[None, :]], 0) * s).astype(np.float32)
    wq1 = np.concatenate([wq1, wq1], 1)          # doubled -> replicated QT
    wk1 = np.concatenate([Wk, bk[None, :]], 0).astype(np.float32)
    wk1 = np.concatenate([wk1, wk1], 1)
    wv1 = np.concatenate([Wv, bv[None, :]], 0).astype(np.float32)
    maps = []
    for core in range(NCORES):
        b, half = core // 2, core % 2
        x1t = np.ascontiguousarray(np.concatenate(
            [x[b], np.ones((L, 1), np.float32)], 1
        ).T.astype(np.float32))              # [65, L]
        xk = np.ascontiguousarray(x1t[:, half * KSH:(half + 1) * KSH])
        maps.append({
            "xt": x1t,
            "xk": xk,
            "wq": wq1, "wk": wk1, "wv": wv1,
        })
    return maps


def _assemble(outs, x):
    full = np.empty((B, L, C), np.float32)
    for b in range(B):
        o0, o1 = outs[2 * b], outs[2 * b + 1]
        att = tuple(
            o0[ph, 0:64] + o0[ph, 64:128] + o1[ph, 0:64] + o1[ph, 64:128]
            for ph in range(4)
        )
        full[b] = np.concatenate(att, axis=1).T + x[b]
    return full


def _run(x, Wq, bq, Wk, bk, Wv, bv, trace=False):
    from concourse.bass_utils import run_bass_kernel_spmd

    nc = _get_nc()
    maps = _in_maps(x, Wq, bq, Wk, bk, Wv, bv)
    res = run_bass_kernel_spmd(
        nc, maps, core_ids=list(range(NCORES)), trace=trace
    )
    outs = [r["o"].astype(np.float32) for r in res.results]
    return _assemble(outs, x), res


def kernel(x, Wq, bq, Wk, bk, Wv, bv):
    x = np.asarray(x, np.float32)
    full, _ = _run(
        x,
        np.asarray(Wq, np.float32), np.asarray(bq, np.float32),
        np.asarray(Wk, np.float32), np.asarray(bk, np.float32),
        np.asarray(Wv, np.float32), np.asarray(bv, np.float32),
    )
    return full


# revision 22
# speedup vs baseline: 1.5079x; 1.0026x over previous
"""Trainium2 Bass kernel for nn_Attention1D (B=4, L=4096, C=64).

reference:
    Q = x@Wq + bq ; K = x@Wk + bk ; V = x@Wv + bv          (per batch b)
    s = Q @ K.T / sqrt(C)                                   [L_q, L_k]
    attn = softmax(s, axis=q)      # normalize over QUERY axis
    out = attn @ V + x

Sharding: 8 cores = 4 batches x 2 key-shards (k in [0,2048) / [2048,4096)).
softmax normalizes over q (not sharded) -> per-core softmax fully local:
    Z[k]   = sum_q exp(s[q,k]);  out_qf = sum_k exp(s[q,k]) * (V[k,f]/Z[k])
k-shards' partial outputs ADD on the host (+ residual x).

The kernel is exp-throughput-bound (8.4M exps/core; ScalarE ACT is the only
native exp at 1/cycle/lane), so the exp is SPLIT across two engines:
  - ScalarE ACT-Exp on q[0, 2048+SS) per k-tile, Z for free via accum_out.
  - VectorE on q[2048+SS, 4096) via the Schraudolph int16 bit-trick:
    i16 = round(s*A + B) bitcast as bf16 == exp(s)*(1+-4%); its Z via one
    tensor_reduce. A = 128/ln2 is folded into Wq host-side (ACT undoes it
    with scale=ln2/128). The trick's sawtooth error washes out in the
    softmax ratio + 4096-key sum + residual (measured 5e-4 total).

PSUM = ONE tile sc4 [128, 4, 1024] (all 8 banks), subtile-dep ring:
  - scores sT[k,q] channel-major: per k-tile, 4 gens each filled by a
    row-packed MM pair (two 512-q chunks concurrently in PE rows 0-63 /
    64-127 via doubled bf16 Q/K channel copies; per-k bf16 score offsets
    cancel in the softmax-over-q ratio). The 4-deep ring decouples the
    PE -> ScalarE/VectorE handoffs; gens 0+1 are consumed by ONE N=2048 ACT.
  - AV runs as a dense post-loop PE stream (HAM-warm K=8/8): outT[f,q]
    accumulated with gv = V/Z stationary (64-col LDWEIGHTS), col-packed
    pairs (even k-tile -> PE cols 0-63 -> rows 0:64, odd -> 64:128,
    per-region start=True), p-major over the 4 freed regions.
Outputs stage bf16 via ScalarE; host does out.T = o[ph][0:64]+o[ph][64:128],
+ partner core + residual. Input DMAs split across sync+gpsimd queues;
gv on GPSIMD; 12 ring warmup MMs overlap the input DMAs.
"""

import numpy as np
import ml_dtypes  # noqa: F401

B, L, C = 4, 4096, 64
NCORES = 8
KSH = L // 2          # keys per core: 2048
NKT = KSH // 128      # 16 k-tiles per core
SS = 768              # ScalarE's share of gen2's 1024 cols (tunable)
HEAT = 0              # heater MMs per score gen (HAM K=8/8 keepalive)
GV_GPSIMD = True      # compute gv = V*rz on GPSIMD (else VectorE)
AEXP = 128.0 / np.log(2.0)          # folded into Wq
BOFF = 16256.0 - 7.42               # int16 exp bias (round-to-nearest HW)

_cache = {}


def _build():
    import concourse.bacc as bacc
    import concourse.mybir as mybir
    import concourse.tile as tile
    from concourse.bass import _add_dep_helper

    bf16 = mybir.dt.bfloat16
    f32 = mybir.dt.float32
    f32r = mybir.dt.float32r
    i16 = mybir.dt.int16
    AF = mybir.ActivationFunctionType
    ALU = mybir.AluOpType
    AX = mybir.AxisListType

    nc = bacc.Bacc("TRN2", target_bir_lowering=False, debug=False)

    xt_d = nc.dram_tensor("xt", [C + 1, L], f32r, kind="ExternalInput")
    xk_d = nc.dram_tensor("xk", [C + 1, KSH], f32r, kind="ExternalInput")
    wq_d = nc.dram_tensor("wq", [C + 1, 2 * C], f32r, kind="ExternalInput")
    wk_d = nc.dram_tensor("wk", [C + 1, 2 * C], f32r, kind="ExternalInput")
    wv_d = nc.dram_tensor("wv", [C + 1, C], f32r, kind="ExternalInput")
    o_d = nc.dram_tensor("o", [4, 128, 1024], bf16, kind="ExternalOutput")

    with tile.TileContext(nc) as tc:
        with (
            tc.tile_pool(name="consts", bufs=1) as consts,
            tc.tile_pool(name="sb", bufs=1) as sb,
            tc.tile_pool(name="scp", bufs=1, space="PSUM") as scp,
        ):
            # ONE PSUM tile: 4 x [128,1024] regions (8 banks), subtile-dep ring
            sc4 = scp.tile([128, 4, 1024], f32, tag="sc4")

            # --- input DMAs first (sync + gpsimd queues in parallel) ---
            wq_s = consts.tile([C + 1, 2 * C], f32r)
            wk_s = consts.tile([C + 1, 2 * C], f32r)
            wv_s = consts.tile([C + 1, C], f32r)
            nc.sync.dma_start(out=wq_s, in_=wq_d.ap())
            nc.gpsimd.dma_start(out=wk_s, in_=wk_d.ap())
            nc.gpsimd.dma_start(out=wv_s, in_=wv_d.ap())

            xt_c = []
            for c in range(8):
                t = sb.tile([C + 1, 512], f32r, tag=f"xt{c}")
                eng = nc.sync if c % 2 == 0 else nc.gpsimd
                eng.dma_start(out=t, in_=xt_d.ap()[:, c * 512:(c + 1) * 512])
                xt_c.append(t)
            xk_c = []
            for c in range(4):
                t = sb.tile([C + 1, 512], f32r, tag=f"xk{c}")
                eng = nc.sync if c % 2 == 0 else nc.gpsimd
                eng.dma_start(out=t, in_=xk_d.ap()[:, c * 512:(c + 1) * 512])
                xk_c.append(t)

            # --- HAM warmup while DMAs stream ---
            wu = consts.tile([128, 512], bf16)
            nc.vector.memset(wu, 0.0)
            for _ in range(12):
                nc.tensor.matmul(sc4[:, 0, 0:512], lhsT=wu[:, 0:128], rhs=wu,
                                 start=True, stop=True)

            # persistent SBUF state
            qt_s = sb.tile([128, 8, 512], bf16, tag="qt")
            kt2 = sb.tile([128, 4, 512], bf16, tag="kt")   # [2C, k-half]
            v_sb = sb.tile([128, NKT, C], bf16, tag="v")
            gv_all = sb.tile([128, NKT, C], bf16, tag="gv")
            e_all = sb.tile([128, NKT * L], bf16, tag="e")
            zps = sb.tile([128, NKT, 3], f32, tag="zps")
            zz = sb.tile([128, NKT], f32, tag="zz")
            rz = sb.tile([128, NKT], f32, tag="rz")
            dume = sb.tile([128, 1], bf16, tag="dume")

            # force the exp table set before any Copy-ACT evacuations
            nc.scalar.activation(out=dume, in_=wu[:, 0:1], func=AF.Exp)

            # --- projections: QT fills all 4 regions, ONE evac ACT ---
            for h in range(8):
                nc.tensor.matmul(
                    sc4[:, h // 2, (h % 2) * 512:(h % 2 + 1) * 512],
                    lhsT=wq_s, rhs=xt_c[h], start=True, stop=True,
                )
                if h == 3:
                    nc.scalar.activation(out=qt_s[:, 0:4, :],
                                         in_=sc4[:, 0:2, :], func=AF.Copy)
            nc.scalar.activation(out=qt_s[:, 4:8, :], in_=sc4[:, 2:4, :],
                                 func=AF.Copy)
            # KT -> regions 0,1 ; V -> region 2
            for h in range(4):
                nc.tensor.matmul(
                    sc4[:, h // 2, (h % 2) * 512:(h % 2 + 1) * 512],
                    lhsT=wk_s, rhs=xk_c[h], start=True, stop=True,
                )
            for kt in range(NKT):
                nc.tensor.matmul(
                    sc4[:, 2, kt * C:(kt + 1) * C],
                    lhsT=xk_c[kt // 4][:, (kt % 4) * 128:(kt % 4 + 1) * 128],
                    rhs=wv_s, start=True, stop=True,
                )
            nc.vector.tensor_copy(out=kt2, in_=sc4[:, 0:2, :])
            nc.vector.tensor_copy(out=v_sb, in_=sc4[:, 2, :])

            def kslice(kt, r0, r1):
                return kt2[r0:r1, kt // 4, (kt % 4) * 128:(kt % 4 + 1) * 128]

            # --- main loop: scores + exp only (AV all post-loop) ---
            for p in range(8):
                for kt in (2 * p, 2 * p + 1):
                    lA = kslice(kt, 0, C)
                    lB = kslice(kt, C, 128)
                    for g in range(4):
                        qc = 2 * g
                        ma = nc.tensor.matmul(
                            sc4[:, g, 0:512], lhsT=lA,
                            rhs=qt_s[0:C, qc, :], tile_position=(0, 0),
                            start=True, stop=True,
                        )
                        mb = nc.tensor.matmul(
                            sc4[:, g, 512:1024], lhsT=lB,
                            rhs=qt_s[C:128, qc + 1, :], tile_position=(C, 0),
                            start=True, stop=True,
                        )
                        _add_dep_helper(mb.ins, ma.ins, sync=False,
                                        reason="score pair order")
                    # one big ACT for q[0,2048), one partial for gen2
                    nc.scalar.activation(
                        out=e_all[:, kt * L:kt * L + 2048], in_=sc4[:, 0:2, :],
                        func=AF.Exp, scale=float(np.log(2.0) / 128.0),
                        accum_out=zps[:, kt, 0:1],
                    )
                    nc.scalar.activation(
                        out=e_all[:, kt * L + 2048:kt * L + 2048 + SS],
                        in_=sc4[:, 2, 0:SS],
                        func=AF.Exp, scale=float(np.log(2.0) / 128.0),
                        accum_out=zps[:, kt, 1:2],
                    )
                    nc.vector.tensor_scalar(
                        out=e_all[:, kt * L + 2048 + SS:kt * L + 3072].bitcast(i16),
                        in0=sc4[:, 2, SS:1024], scalar1=BOFF,
                        scalar2=None, op0=ALU.add,
                    )
                    nc.vector.tensor_scalar(
                        out=e_all[:, kt * L + 3072:(kt + 1) * L].bitcast(i16),
                        in0=sc4[:, 3, :], scalar1=BOFF,
                        scalar2=None, op0=ALU.add,
                    )
                    nc.vector.tensor_reduce(
                        out=zps[:, kt, 2:3],
                        in_=e_all[:, kt * L + 2048 + SS:(kt + 1) * L],
                        axis=AX.X, op=ALU.add,
                    )
                if p % 2 == 1:
                    j = 4 * (p // 2)
                    nc.vector.tensor_reduce(
                        out=zz[:, j:j + 4], in_=zps[:, j:j + 4, :],
                        axis=AX.X, op=ALU.add,
                    )
                    nc.vector.reciprocal(out=rz[:, j:j + 4], in_=zz[:, j:j + 4])
                    for kt in range(j, j + 4):
                        if GV_GPSIMD:
                            nc.gpsimd.tensor_scalar(
                                out=gv_all[:, kt, :], in0=v_sb[:, kt, :],
                                scalar1=rz[:, kt:kt + 1], scalar2=None,
                                op0=ALU.mult,
                            )
                        else:
                            nc.vector.tensor_scalar_mul(
                                gv_all[:, kt, :], v_sb[:, kt, :],
                                rz[:, kt:kt + 1]
                            )

            # --- AV: dense post-loop stream, p-major over 4 phase regions ---
            for p in range(8):
                ke, ko = 2 * p, 2 * p + 1
                for ph in range(4):
                    for cq in range(2):
                        qg = ph * 1024 + cq * 512
                        me = nc.tensor.matmul(
                            sc4[0:64, ph, cq * 512:(cq + 1) * 512],
                            lhsT=gv_all[:, ke, :],
                            rhs=e_all[:, ke * L + qg:ke * L + qg + 512],
                            tile_position=(0, 0),
                            start=(p == 0), stop=(p == 7),
                            skip_group_check=True,
                        )
                        mo = nc.tensor.matmul(
                            sc4[64:128, ph, cq * 512:(cq + 1) * 512],
                            lhsT=gv_all[:, ko, :],
                            rhs=e_all[:, ko * L + qg:ko * L + qg + 512],
                            tile_position=(0, 64),
                            start=(p == 0), stop=(p == 7),
                            skip_group_check=True,
                        )
                        _add_dep_helper(mo.ins, me.ins, sync=False,
                                        reason="av pair order")
            for ph in range(4):
                ob1 = sb.tile([128, 1024], bf16, tag=f"ob{ph}")
                nc.scalar.activation(out=ob1, in_=sc4[:, ph, :], func=AF.Copy)
                nc.sync.dma_start(out=o_d.ap()[ph], in_=ob1)

    nc.compile()
    return nc


def _get_nc():
    if "nc" not in _cache:
        _cache["nc"] = _build()
    return _cache["nc"]


def _in_maps(x, Wq, bq, Wk, bk, Wv, bv):
    s = np.float32(AEXP / np.sqrt(np.float32(C)))
    wq1 = (np.concatenate([Wq, bq[None, :]], 0) * s).astype(np.float32)
    wq1 = np.concatenate([wq1, wq1], 1)          # doubled -> replicated QT
    wk1 = np.concatenate([Wk, bk[None, :]], 0).astype(np.float32)
    wk1 = np.concatenate([wk1, wk1], 1)
    wv1 = np.concatenate([Wv, bv[None, :]], 0).astype(np.float32)
    maps = []
    for core in range(NCORES):
        b, half = core // 2, core % 2
        x1t = np.ascontiguousarray(np.concatenate(
            [x[b], np.ones((L, 1), np.float32)], 1
        ).T.astype(np.float32))              # [65, L]
        xk = np.ascontiguousarray(x1t[:, half * KSH:(half + 1) * KSH])
        maps.append({
            "xt": x1t,
            "xk": xk,
            "wq": wq1, "wk": wk1, "wv": wv1,
        })
    return maps


def _assemble(outs, x):
    full = np.empty((B, L, C), np.float32)
    for b in range(B):
        o0, o1 = outs[2 * b], outs[2 * b + 1]
        att = tuple(
            o0[ph, 0:64] + o0[ph, 64:128] + o1[ph, 0:64] + o1[ph, 64:128]
            for ph in range(4)
        )
        full[b] = np.concatenate(att, axis=1).T + x[b]
    return full


def _run(x, Wq, bq, Wk, bk, Wv, bv, trace=False):
    from concourse.bass_utils import run_bass_kernel_spmd

    nc = _get_nc()
    maps = _in_maps(x, Wq, bq, Wk, bk, Wv, bv)
    res = run_bass_kernel_spmd(
        nc, maps, core_ids=list(range(NCORES)), trace=trace
    )
    outs = [r["o"].astype(np.float32) for r in res.results]
    return _assemble(outs, x), res


def kernel(x, Wq, bq, Wk, bk, Wv, bv):
    x = np.asarray(x, np.float32)
    full, _ = _run(
        x,
        np.asarray(Wq, np.float32), np.asarray(bq, np.float32),
        np.asarray(Wk, np.float32), np.asarray(bk, np.float32),
        np.asarray(Wv, np.float32), np.asarray(bv, np.float32),
    )
    return full
